# revision 23
# baseline (speedup 1.0000x reference)
"""Trainium2 Bass kernel for nn_ChannelSpatialContextAttention.

Sharding: pure data-parallel - batch B=8, one image per NeuronCore.

Per core (batch dim dropped): x [512, 16384] -> y [512, 16384].
All grouped 1x1 convs are tiny channel mixes; block-diagonalized and
algebraically fused on the host:

    xc      = relu(inorm(Wc @ x))                       compress 512->32
    att_pre = A1eff @ xc        (A1eff = A1[:, :32]@K + A1[:, 32:])
    att     = relu(inorm(att_pre))
    logits  = A2 @ att
    sm*lam  = exp(logits/tau) * (lam * S / S_c)         max-free softmax
    amv     = sm*lam*v + (1-lam)*sigmoid(logits)*v      v = V @ xc
    coord   = (alpha*ah(c,h) + beta*aw(c,w)) * xc       coord attention
    fused   = relu(inorm(Fc@coord + (M@K)@xc + 0.15M@p2 + Msm@p1))
    gate    = sigmoid(gw . mean(fused)) * 0.95 + 0.05
    y       = E @ fused * gate                          expand 32->512

Layout: S-sized tensors in SBUF as [128, 4096], partition p = chunk*32+c
(4 spatial chunks of 4096).  IO is bf16 both ways (host converts): halves
HBM traffic; all big matmuls use bf16 lhsT x bf16 rhs (walrus forbids
mixing 32-bit with 16-bit operands) at 1 cyc/row.  Compress packs 4
chunk-blocks into full [128,1024] PSUM tiles via PSUM column offsets so
stats/copies run on 128 partitions.  Cross-chunk sums -> J matmul.

Pipeline structure (2-deep software pipeline; reps overlap):
- input DMAs on SP queue, ring-split patch pins SP DMAs to HWDGE rings
  0-3 and ACT DMAs (outputs + small) to 4-7 so ring completion counters
  don't chain rep k+1's inputs behind rep k's outputs.
- phase A (input DMA + compress) of rep k+1 is woven into rep k's mid
  phases at P0..P4; P0 sits at the rep top so the first input DMA
  anchors right after the previous rep's gate.
- the EXPAND of rep k is deferred: its 16 (tg,mc) groups + dma flush
  are emitted at ~19 EW() weave points inside rep k+1's mid phases, so
  the output stream + PSUM evacuation overlap the next rep's compute
  instead of serializing at the rep tail.  One yb = [128,4096] bf16 ->
  one 1MiB DMA, issued one group LATE so its sem waits are satisfied
  (no ACT.SEQ head-of-line stall).  Gated copies split ACT:DVE 38:26
  to time-balance both engines.
- barrier chains (inorm scale/bias via J-matmul cross-chunk combine,
  softmax esum, gate) run at scheduler priority 0 so their tiny ops
  never queue behind woven bulk work; chains use 1 Newton rsqrt
  iteration (2 for the output-scaling fused inorm), host-negated
  gammas, 0.25-scaled jm2, and lam*S folded into KR_M.
- the coord branch runs pair-vectorized (h and w stacked [*, 2, 128]) on
  replicated 128-partition tiles (host-tiled lhsT), softmax without max
  subtraction; wlin on the idle Pool engine.
"""

import os
import numpy as np
import ml_dtypes

import concourse.bass as bass
import concourse.tile as tile
import concourse.mybir as mybir
from concourse.bass_utils import run_bass_kernel_spmd

try:  # persistent NEFF compile cache across calls/processes (best effort)
    import jax
    jax.config.update("jax_compilation_cache_dir", "/tmp/jax_cc_cache")
    jax.config.update("jax_persistent_cache_min_compile_time_secs", 0)
except Exception:
    pass

dt = mybir.dt
AF = mybir.ActivationFunctionType
ALU = mybir.AluOpType
AX = mybir.AxisListType

NCORES = 8
C_IN = 512
M = 32
G = 4
MG = M // G
H = 128
W = 128
S = H * W
TS = 512
NCH = 4
CHS = S // NCH
EPS = 1e-5
COT_TAU = 0.8
COT_LAM = 0.7
GATE_FLOOR = 0.05

f32 = dt.float32
f32r = dt.float32r
bf16 = dt.bfloat16

KR_A1EFF, KR_A2, KR_V, KR_MK, KR_M = range(5)
KB_FC, KB_M15 = range(2)
PP_INCG, PP_INCB, PP_ATTG, PP_ATTB, PP_FUSG, PP_FUSB, PP_GW = range(7)

_RING_PATCHED = False


def _patch_ring_split():
    """Pin SP-issued HWDGE DMAs to rings 0-3 and other engines' to 4-7.

    The stock round-robin mixes input and output DMAs on the same rings;
    ring completion sems are counters, so a WAW dep on an input DMA's
    ring tick transitively waits for every earlier DMA on that ring --
    including the previous rep's output stream. Splitting the rings
    breaks that cross-coupling.
    """
    global _RING_PATCHED
    if _RING_PATCHED:
        return
    _RING_PATCHED = True
    import concourse.tile_sem_assignment as tsa
    from concourse.tile_scheduler import DMAInst
    from concourse import bass_isa

    orig = tsa.TileClockTick._assign_tick

    def _assign_tick_split(self, inst):
        if (isinstance(inst, DMAInst)
                and not isinstance(inst, bass_isa.UserSyncedRemoteDMADescs)
                and inst.engine != mybir.EngineType.Pool):
            cnts = getattr(self, "_ring_cnts", None)
            if cnts is None:
                cnts = self._ring_cnts = {}
            grp = 0 if inst.engine == mybir.EngineType.SP else 1
            k = cnts.get(grp, 0)
            cnts[grp] = k + 1
            self.next_hw_dma_idx = grp * 4 + (k % 4)
        return orig(self, inst)

    tsa.TileClockTick._assign_tick = _assign_tick_split


def _block_diag(w):
    g, o, i = w.shape
    out = np.zeros((g * o, g * i), np.float32)
    for k in range(g):
        out[k * o:(k + 1) * o, k * i:(k + 1) * i] = w[k]
    return out


def _kron128(w):
    """W [32,32] (out,in) -> lhsT [128,128] = kron(I4, W.T)."""
    return np.kron(np.eye(NCH, dtype=np.float32),
                   np.ascontiguousarray(w.T, dtype=np.float32))


def _newton_rsqrt(eng, pool, y_out, v_in, shape, tagp, iters=2):
    """y_out sbuf fp32 = rsqrt(v_in) elementwise; no ACT table needed."""
    yi = pool.tile(shape, dt.int32, tag="nri" + tagp)
    eng.tensor_single_scalar(yi[:], v_in.bitcast(dt.int32), 1,
                             ALU.logical_shift_right)
    eng.tensor_scalar(yi[:], yi[:], -1, 0x5F3759DF, ALU.mult, ALU.add)
    y = yi[:].bitcast(f32)
    half = pool.tile(shape, f32, tag="nrh" + tagp)
    eng.tensor_scalar_mul(half[:], v_in, 0.5)
    t = pool.tile(shape, f32, tag="nrt" + tagp)
    for _ in range(iters):
        eng.tensor_tensor(t[:], y, y, ALU.mult)
        eng.tensor_tensor(t[:], half[:], t[:], ALU.mult)
        eng.tensor_scalar(t[:], t[:], -1.0, 1.5, ALU.mult, ALU.add)
        eng.tensor_tensor(y, y, t[:], ALU.mult)
    eng.tensor_copy(y_out, y)


def _inorm_scale_bias(nc, pool, pj_mean, pj_e2, g_ap, b_ap, tagp):
    """From J-combined [mean, E2] ([128,1] sbuf aps): returns (scale, bias)
    [128,1] sbuf aps for relu(x*scale+bias)."""
    eng = nc.vector
    var = pool.tile([128, 1], f32, tag="inv" + tagp)
    eng.tensor_tensor(var[:], pj_mean, pj_mean, ALU.mult)
    eng.tensor_tensor(var[:], pj_e2, var[:], ALU.subtract)
    eng.tensor_scalar_add(var[:], var[:], EPS)
    rs = pool.tile([128, 1], f32, tag="inr" + tagp)
    _newton_rsqrt(eng, pool, rs[:], var[:], [128, 1], tagp)
    scl = pool.tile([128, 1], f32, tag="ins" + tagp)
    eng.tensor_tensor(scl[:], rs[:], g_ap, ALU.mult)
    nscl = pool.tile([128, 1], f32, tag="inn" + tagp)
    eng.tensor_scalar_mul(nscl[:], scl[:], -1.0)
    bia = pool.tile([128, 1], f32, tag="inb" + tagp)
    eng.tensor_tensor(bia[:], pj_mean, nscl[:], ALU.mult)
    eng.tensor_tensor(bia[:], bia[:], b_ap, ALU.add)
    return scl, bia


def split_multi_waits(nc):
    """This env's walrus supports at most one sync-wait per instruction:
    hoist extra waits onto same-engine NOPs inserted just before."""
    for f in nc.m.functions:
        for bb in f.blocks:
            il = bb.instructions
            out = []
            dirty = False
            for ins in il:
                si = ins.sync_info
                waits = list(si.on_wait) if si is not None else []
                if len(waits) > 1:
                    dirty = True
                    for k, w in enumerate(waits[:-1]):
                        nop = mybir.InstNoOp(
                            name=f"wsplit_{ins.name}_{k}", ins=[], outs=[])
                        nop.engine = ins.engine
                        nop.sync_info = mybir.SyncInfo(on_wait=[w],
                                                       on_update=[])
                        out.append(nop)
                    ins.sync_info = mybir.SyncInfo(
                        on_wait=[waits[-1]], on_update=list(si.on_update))
                out.append(ins)
            if dirty:
                bb.instructions = out


def _agg_c2(nc, pool, st, tagp):
    """bn_stats buffer [128, n, 6] -> c2 [128,2] = [mean/4, (var+mean^2)/4]
    (scaled so a J-sum over the 4 chunks yields full-channel mean/E2)."""
    ag = pool.tile([128, 2], f32, tag="ag" + tagp)
    nc.vector.bn_aggr(ag[:], st[:])
    c2 = pool.tile([128, 2], f32, tag="c2" + tagp)
    nc.vector.tensor_scalar_mul(c2[:, 0:1], ag[:, 0:1], 0.25)
    e2 = pool.tile([128, 1], f32, tag="e2" + tagp)
    nc.vector.scalar_tensor_tensor(e2[:], ag[:, 0:1], ag[:, 0:1], ag[:, 1:2],
                                   ALU.mult, ALU.add)
    nc.vector.tensor_scalar_mul(c2[:, 1:2], e2[:], 0.25)
    return c2


def build_program(debug=False, reps=1):
    _patch_ring_split()
    nc = bass.Bass("TRN2", target_bir_lowering=False, debug=False,
                   num_devices=NCORES)

    x_e = nc.dram_tensor("x", [C_IN, S], bf16, kind="ExternalInput")
    cw_e = nc.dram_tensor("cw", [128, 4, 32], bf16, kind="ExternalInput")
    ew_e = nc.dram_tensor("ew", [128, 512], f32, kind="ExternalInput")
    kr_e = nc.dram_tensor("kr", [128, 5, 128], f32, kind="ExternalInput")
    kb_e = nc.dram_tensor("kb", [128, 2, 128], bf16, kind="ExternalInput")
    jm_e = nc.dram_tensor("jm", [128, 128], f32, kind="ExternalInput")
    cm_e = nc.dram_tensor("cm", [32, 32], f32, kind="ExternalInput")
    c2_e = nc.dram_tensor("c2m", [32, 2, 128], f32, kind="ExternalInput")
    pp_e = nc.dram_tensor("pp", [128, 8], f32, kind="ExternalInput")
    cp_e = nc.dram_tensor("cp", [32, 4], f32, kind="ExternalInput")
    cc_e = nc.dram_tensor("cc", [128, 2], f32, kind="ExternalInput")
    on_e = nc.dram_tensor("on", [1, 128], f32, kind="ExternalInput")
    y_e = nc.dram_tensor("y", [C_IN, S], bf16, kind="ExternalOutput")
    dbg = {}
    if debug:
        for nm in ["d_xc", "d_coord", "d_p1", "d_p2", "d_fused"]:
            dbg[nm] = nc.dram_tensor(nm, [128, CHS], bf16,
                                     kind="ExternalOutput")
        dbg["d_small"] = nc.dram_tensor("d_small", [128, 16], f32,
                                        kind="ExternalOutput")
        dbg["d_ahw"] = nc.dram_tensor("d_ahw", [128, 2, 128], bf16,
                                      kind="ExternalOutput")

    with tile.TileContext(nc) as tc:
      with tc.tile_pool(name="wpool", bufs=1) as wp, \
           tc.tile_pool(name="stream", bufs=5) as strm, \
           tc.tile_pool(name="big", bufs=1) as big, \
           tc.tile_pool(name="xcr", bufs=2) as xcr, \
           tc.tile_pool(name="afp", bufs=1) as afp, \
           tc.tile_pool(name="thp", bufs=2) as thp, \
           tc.tile_pool(name="wlinp", bufs=1) as wlp, \
           tc.tile_pool(name="crdp", bufs=2) as crdp, \
           tc.tile_pool(name="ybuf", bufs=2) as ybp, \
           tc.tile_pool(name="small", bufs=1) as sm, \
           tc.tile_pool(name="psA", bufs=2, space="PSUM") as psA, \
           tc.tile_pool(name="psE", bufs=2, space="PSUM") as psE:
        # ------------- weights / params (loaded once, SP queue) ------
        cw = wp.tile([128, 4, 32], bf16, tag="cw")
        nc.sync.dma_start(cw[:], cw_e.ap())
        ew = wp.tile([128, 512], f32r, tag="ew")
        nc.sync.dma_start(ew[:], ew_e.ap().bitcast(f32r))
        kr = wp.tile([128, 5, 128], f32r, tag="kr")
        nc.sync.dma_start(kr[:], kr_e.ap().bitcast(f32r))
        kb = wp.tile([128, 2, 128], bf16, tag="kb")
        nc.sync.dma_start(kb[:], kb_e.ap())
        jm = wp.tile([128, 128], f32, tag="jm")
        nc.sync.dma_start(jm[:], jm_e.ap())
        cm = wp.tile([32, 32], f32r, tag="cm")
        nc.sync.dma_start(cm[:], cm_e.ap().bitcast(f32r))
        cm2 = wp.tile([32, 2, 128], f32r, tag="cm2")
        nc.sync.dma_start(cm2[:], c2_e.ap().bitcast(f32r))
        pp = wp.tile([128, 8], f32, tag="pp")
        nc.sync.dma_start(pp[:], pp_e.ap())
        cp = wp.tile([32, 4], f32, tag="cp")
        nc.sync.dma_start(cp[:], cp_e.ap())
        cc = wp.tile([128, 2], f32, tag="cc")
        nc.sync.dma_start(cc[:], cc_e.ap())
        ones_t = wp.tile([1, 128], f32, tag="ones_t")
        nc.sync.dma_start(ones_t[:], on_e.ap())

        x_r = x_e.ap().rearrange("(kc p) s -> p kc s", p=128)

        # ---- software-pipelined phase A (compress) ----
        # rep k+1's input DMAs + compress quarters are woven between rep
        # k's phases (DMA for quarter j issues one weave point before its
        # compress) so the input stream spreads across the whole rep and
        # compress matmuls fill PE barrier stalls.
        def phaseA_alloc():
            return {"xcraw": xcr.tile([128, CHS], f32, tag="xcraw",
                                      name="xcraw"),
                    "stA": sm.tile([128, 8, 6], f32, tag="stA",
                                   name="stA"),
                    "xins": {}}

        def phaseA_dma(st, j):
            xins = []
            for ch in range(NCH):
                xin = strm.tile([128, 4, 1024], bf16, tag="xin")
                nc.sync.dma_start(
                    xin[:],
                    x_r[:, :, ch * CHS + j * 1024:
                        ch * CHS + (j + 1) * 1024])
                xins.append(xin)
            st["xins"][j] = xins

        def phaseA_compute(st, j):
            xins = st["xins"].pop(j)
            ps = psA.tile([128, 1024], f32, tag="mm")
            for ch in range(NCH):
                for kc in range(4):
                    for h in range(2):
                        nc.tensor.matmul(
                            ps[ch * 32:(ch + 1) * 32,
                               h * TS:(h + 1) * TS],
                            cw[:, kc, :],
                            xins[ch][:, kc, h * TS:(h + 1) * TS],
                            start=(kc == 0), stop=(kc == 3),
                            tile_position=(0, ch * 32))
            for h in range(2):
                nc.vector.bn_stats(st["stA"][:, j * 2 + h, :],
                                   ps[:, h * TS:(h + 1) * TS])
            nc.scalar.copy(st["xcraw"][:, j * 1024:(j + 1) * 1024], ps[:])

        # ---- expand phase of rep k, emitted as 17 deferred items (16
        # (tg,mc) groups + dma flush) woven into rep k+1's mid phases so
        # the output stream + PSUM evacuation overlap the next rep's
        # compute instead of serializing at the rep tail. ----
        def _make_expand(fused_t, gate_t):
            st = {"pending": None, "ei": 0}

            def mk(tg, mc):
                def g():
                    yb = ybp.tile([128, 4096], bf16, tag="yb", name="yb")
                    for k in range(4):
                        ps2 = psE.tile([128, 1024], f32, tag="me",
                                       name="ps2")
                        for h in range(2):
                            nc.tensor.matmul(
                                ps2[:, h * TS:(h + 1) * TS],
                                ew[tg * 32:(tg + 1) * 32,
                                   mc * 128:(mc + 1) * 128],
                                fused_t[tg * 32:(tg + 1) * 32,
                                        k * 1024 + h * TS:
                                        k * 1024 + (h + 1) * TS],
                                start=True, stop=True,
                                tile_position=(tg * 32, 0))
                        dstp = yb[:, k * 1024:(k + 1) * 1024]
                        ei = st["ei"]
                        st["ei"] = ei + 1
                        # 23/64 of the gated copies go to DVE (time-balances
                        # ACT@1.038us vs DVE@1.192us per [128,1024] copy)
                        if (ei + 1) * 23 // 64 > ei * 23 // 64:
                            nc.vector.tensor_scalar_mul(dstp, ps2[:],
                                                        gate_t[:])
                        else:
                            nc.scalar.mul(dstp, ps2[:], gate_t[:])
                        if k == 0 and st["pending"] is not None:
                            nc.scalar.dma_start(*st["pending"])
                            st["pending"] = None
                    st["pending"] = (y_e.ap()[mc * 128:(mc + 1) * 128,
                                              tg * 4096:(tg + 1) * 4096],
                                     yb[:])
                return g

            items = [mk(tg, mc) for tg in range(4) for mc in range(4)]

            def flush():
                if st["pending"] is not None:
                    nc.scalar.dma_start(*st["pending"])
                    st["pending"] = None

            items.append(flush)
            return items

        exp_items = []

        def EW():
            if exp_items:
                exp_items.pop(0)()

        # prologue: rep 0's phase A runs un-pipelined
        cur = phaseA_alloc()
        for j in range(4):
            phaseA_dma(cur, j)
            phaseA_compute(cur, j)

        for rep_i in range(reps):
            nxt = None
            xcraw, stA = cur["xcraw"], cur["stA"]
            # ------------- xc inorm + relu -------------
            c2a = _agg_c2(nc, sm, stA[:], "a")
            pja = psA.tile([128, 1024], f32, tag="mm")
            nc.tensor.matmul(pja[:, 0:2], jm[:], c2a[:], start=True, stop=True)
            pja_s = sm.tile([128, 2], f32)
            nc.vector.tensor_copy(pja_s[:], pja[:, 0:2])
            scl1, bia1 = _inorm_scale_bias(nc, sm, pja_s[:, 0:1], pja_s[:, 1:2],
                                           pp[:, PP_INCG:PP_INCG + 1],
                                           pp[:, PP_INCB:PP_INCB + 1], "1")
            if rep_i + 1 < reps:                       # weave P0
                nxt = phaseA_alloc()
                phaseA_dma(nxt, 0)
            # xc relu + early coord reductions + attpre, interleaved per
            # chunk so the coord latency chain starts ASAP while the PE
            # stream runs attpre with no coord tinies in front of it.
            xc = big.tile([128, CHS], f32r, tag="xc")
            xcf = xc[:].bitcast(f32)
            zhp = sm.tile([128, 32], f32)
            zwp4 = sm.tile([128, 4, W], f32)
            attpre = afp.tile([128, CHS], f32, tag="af")
            stB = sm.tile([128, 8, 6], f32)
            for q in range(4):
                sl = slice(q * 1024, (q + 1) * 1024)
                nc.scalar.activation(xc[:, sl], xcraw[:, sl], AF.Relu,
                                     bias=bia1[:], scale=scl1[:])
                nc.vector.tensor_reduce(
                    zhp[:, q * 8:(q + 1) * 8],
                    xcf[:, sl].rearrange("p (a b) -> p a b", b=W),
                    axis=AX.X, op=ALU.add)
                nc.vector.tensor_reduce(
                    zwp4[:, q, :],
                    xcf[:, sl].rearrange("p (a b) -> p b a", b=W),
                    axis=AX.X, op=ALU.add)
                ps = psE.tile([128, 1024], f32, tag="me")
                for h in range(2):
                    nc.tensor.matmul(ps[:, h * TS:(h + 1) * TS],
                                     kr[:, KR_A1EFF, :],
                                     xc[:, q * 1024 + h * TS:
                                        q * 1024 + (h + 1) * TS],
                                     start=True, stop=True)
                    nc.vector.bn_stats(stB[:, q * 2 + h, :],
                                       ps[:, h * TS:(h + 1) * TS])
                nc.vector.tensor_copy(attpre[:, sl], ps[:])
            if nxt:                                    # weave P1
                phaseA_compute(nxt, 0)
                phaseA_dma(nxt, 1)
            # cross-chunk combine for zw (DVE) + zh assembly into zc
            zwp = sm.tile([128, W], f32)
            nc.vector.tensor_reduce(
                zwp[:], zwp4[:].rearrange("p q w -> p w q"),
                axis=AX.X, op=ALU.add)
            zhps = sm.tile([128, 32], f32)
            nc.vector.tensor_scalar_mul(zhps[:], zhp[:], 1.0 / W)
            zc = sm.tile([32, 2, 128], f32r)
            for ch in range(NCH):
                nc.scalar.dma_start(zc[:, 0, ch * 32:(ch + 1) * 32],
                                    zhps[ch * 32:(ch + 1) * 32, :]
                                    .bitcast(f32r))

            # ------------- att inorm + relu -------------
            c2b = _agg_c2(nc, sm, stB[:], "b")
            pjb = psA.tile([128, 1024], f32, tag="mm")
            nc.tensor.matmul(pjb[:, 0:2], jm[:], c2b[:], start=True, stop=True)
            pjb_s = sm.tile([128, 2], f32)
            nc.vector.tensor_copy(pjb_s[:], pjb[:, 0:2])
            scl2, bia2 = _inorm_scale_bias(nc, sm, pjb_s[:, 0:1], pjb_s[:, 1:2],
                                           pp[:, PP_ATTG:PP_ATTG + 1],
                                           pp[:, PP_ATTB:PP_ATTB + 1], "2")
            # ------- logits + v + tanh/exp + p1/p2 per 1024-chunk -------
            # tanh/exp read the logits PSUM tile directly (no SBUF staging).
            # p1 = exp_t * v (softmax scale folded into krs lhsT)
            # p2 = (tanh(logits/2) + 1) * v (0.15 folded into M15 lhsT)
            # max-free softmax: post-inorm logits are bounded well inside
            # fp32 exp range, so no max subtraction barrier is needed.
            esum4 = sm.tile([128, 4], f32)
            p1 = big.tile([128, CHS], bf16, tag="p1")
            p2 = big.tile([128, CHS], bf16, tag="p2")
            for q in range(4):
                sl = slice(q * 1024, (q + 1) * 1024)
                atq = thp.tile([128, 1024], f32r, tag="att")
                nc.scalar.activation(atq[:], attpre[:, sl], AF.Relu,
                                     bias=bia2[:], scale=scl2[:])
                lps = psA.tile([128, 1024], f32, tag="mm")
                for h in range(2):
                    nc.tensor.matmul(lps[:, h * TS:(h + 1) * TS],
                                     kr[:, KR_A2, :],
                                     atq[:, h * TS:(h + 1) * TS],
                                     start=True, stop=True)
                psv = psE.tile([128, 1024], f32, tag="me")
                for h in range(2):
                    nc.tensor.matmul(psv[:, h * TS:(h + 1) * TS],
                                     kr[:, KR_V, :],
                                     xc[:, q * 1024 + h * TS:
                                        q * 1024 + (h + 1) * TS],
                                     start=True, stop=True)
                th_t = thp.tile([128, 1024], bf16, tag="th")
                exp_t = thp.tile([128, 1024], bf16, tag="exp")
                nc.scalar.activation(th_t[:], lps[:], AF.Tanh,
                                     bias=0.0, scale=0.5)
                nc.scalar.activation(exp_t[:], lps[:], AF.Exp,
                                     bias=0.0, scale=1.0 / COT_TAU,
                                     accum_out=esum4[:, q:q + 1])
                nc.vector.tensor_tensor(p1[:, sl], exp_t[:], psv[:],
                                        ALU.mult)
                nc.vector.scalar_tensor_tensor(p2[:, sl], th_t[:],
                                               1.0, psv[:],
                                               ALU.add, ALU.mult)

            if nxt:                                    # weave P2
                phaseA_compute(nxt, 1)
                phaseA_dma(nxt, 2)
            esumS = sm.tile([128, 1], f32)
            nc.vector.tensor_reduce(esumS[:], esum4[:], axis=AX.X, op=ALU.add)
            pS = psA.tile([128, 1024], f32, tag="mm")
            nc.tensor.matmul(pS[:, 0:1], jm[:], esumS[:], start=True,
                             stop=True)
            recS = sm.tile([128, 1], f32)
            nc.vector.reciprocal(recS[:], pS[:, 0:1])
            smscl = sm.tile([128, 1], f32)
            nc.vector.tensor_scalar_mul(smscl[:], recS[:], COT_LAM * S)
            # krs = M_kron * smscl (per-contraction-row scale)
            krs = wp.tile([128, 128], bf16, tag="krs")
            nc.vector.tensor_scalar_mul(krs[:], kr[:, KR_M, :].bitcast(f32),
                                        smscl[:])
            if debug:
                nc.scalar.dma_start(dbg["d_p1"].ap(), p1[:])
                nc.scalar.dma_start(dbg["d_p2"].ap(), p2[:])
                nc.scalar.dma_start(dbg["d_small"].ap()[:, 7:8], smscl[:])

            # ---- coord tinies, pair-vectorized (h|w stacked on free dim),
            # late in PE order so they never stall the main PE stream ----
            pzw = psE.tile([128, 1024], f32, tag="me")
            nc.tensor.matmul(pzw[:, 0:W], jm[:], zwp[:], start=True, stop=True)
            nc.vector.tensor_scalar_mul(zc[:, 1, :], pzw[0:32, 0:W], 1.0 / H)
            # proj = silu(inorm(P @ [zh|zw]))
            ppj = psE.tile([128, 1024], f32, tag="me")
            nc.tensor.matmul(ppj[0:32, 0:256],
                             cm[:], zc[:].rearrange("p a b -> p (a b)"),
                             start=True, stop=True)
            ppj_s = sm.tile([32, 2, 128], f32)
            nc.vector.tensor_copy(ppj_s[:], ppj[0:32, 0:256]
                                  .rearrange("p (a b) -> p a b", b=128))
            stp = sm.tile([32, 2, 6], f32)
            agp = sm.tile([32, 2, 2], f32)
            for k in range(2):
                nc.vector.bn_stats(stp[:, k, :], ppj_s[:, k, :])
                nc.vector.bn_aggr(agp[:, k, :], stp[:, k, :])
            vp = sm.tile([32, 2], f32)
            nc.vector.tensor_scalar_add(vp[:], agp[:, :, 1], EPS)
            rsp = sm.tile([32, 2], f32)
            _newton_rsqrt(nc.vector, sm, rsp[:], vp[:], [32, 2], "c")
            sclp = sm.tile([32, 2], f32)
            nc.vector.tensor_tensor(sclp[:], rsp[:], cp[:, 0:2], ALU.mult)
            nsclp = sm.tile([32, 2], f32)
            nc.vector.tensor_scalar_mul(nsclp[:], sclp[:], -1.0)
            biap = sm.tile([32, 2], f32)
            nc.vector.tensor_tensor(biap[:], agp[:, :, 0], nsclp[:], ALU.mult)
            nc.vector.tensor_tensor(biap[:], biap[:], cp[:, 2:4], ALU.add)
            ut = sm.tile([32, 2, 128], f32)
            nc.vector.tensor_tensor(ut[:], ppj_s[:],
                                    sclp[:].unsqueeze(2)
                                    .broadcast_to([32, 2, 128]), ALU.mult)
            nc.vector.tensor_tensor(ut[:], ut[:],
                                    biap[:].unsqueeze(2)
                                    .broadcast_to([32, 2, 128]), ALU.add)
            # silu(u) = u * (0.5 + 0.5*tanh(u/2))
            sg = sm.tile([32, 2, 128], f32)
            nc.scalar.activation(sg[:], ut[:], AF.Tanh, bias=0.0, scale=0.5)
            nc.vector.tensor_scalar(sg[:], sg[:], 0.5, 0.5, ALU.mult, ALU.add)
            proj = sm.tile([32, 2, 128], f32r)
            nc.vector.tensor_tensor(proj[:], sg[:], ut[:], ALU.mult)
            # z_h/z_w projections, replicated out to 128 partitions via
            # host-tiled lhsT; then pairwise mean-rescaled softmax, no max
            # subtraction (post-inorm values are tiny).
            pzf = psE.tile([128, 1024], f32, tag="me")
            for k in range(2):
                nc.tensor.matmul(pzf[:, k * 128:(k + 1) * 128],
                                 cm2[:, k, :], proj[:, k, :],
                                 start=True, stop=True)
            ex = sm.tile([128, 2, 128], f32)
            nc.scalar.activation(ex[:], pzf[:, 0:256]
                                 .rearrange("p (a b) -> p a b", b=128),
                                 AF.Exp, bias=0.0, scale=1.0)
            es = sm.tile([128, 2], f32)
            nc.vector.tensor_reduce(es[:], ex[:], axis=AX.X, op=ALU.add)
            escl = sm.tile([128, 2], f32)
            nc.vector.reciprocal(escl[:], es[:])
            nc.vector.tensor_tensor(escl[:], escl[:], cc[:], ALU.mult)
            a_s = sm.tile([128, 2, 128], bf16)
            nc.vector.tensor_tensor(a_s[:], ex[:],
                                    escl[:].unsqueeze(2)
                                    .broadcast_to([128, 2, 128]), ALU.mult)
            ah128b = sm.tile([128, 32], bf16)
            for ch in range(NCH):
                nc.vector.tensor_copy(
                    ah128b[ch * 32:(ch + 1) * 32, :],
                    a_s[ch * 32:(ch + 1) * 32, 0,
                        ch * 32:(ch + 1) * 32])
            if nxt:                                    # weave P3
                phaseA_compute(nxt, 2)
                phaseA_dma(nxt, 3)

            # ------------- fused (wlin/coord built per quarter) -----------
            fraw = afp.tile([128, CHS], f32, tag="af")
            stF = sm.tile([128, 8, 6], f32)
            for q in range(4):
                sl = slice(q * 1024, (q + 1) * 1024)
                wlin = wlp.tile([128, 8, W], bf16, tag="wlin")
                nc.gpsimd.tensor_tensor(
                    wlin[:],
                    ah128b[:, q * 8:(q + 1) * 8].unsqueeze(2)
                    .broadcast_to([128, 8, W]),
                    a_s[:, 1, :].unsqueeze(1).broadcast_to([128, 8, W]),
                    ALU.add)
                coord = crdp.tile([128, 1024], bf16, tag="coord")
                nc.gpsimd.tensor_tensor(coord[:],
                                        wlin[:].rearrange("p a b -> p (a b)"),
                                        xcf[:, sl], ALU.mult)
                ps = psA.tile([128, 1024], f32, tag="mm")
                for h in range(2):
                    hsl = slice(q * 1024 + h * TS, q * 1024 + (h + 1) * TS)
                    osl = slice(h * TS, (h + 1) * TS)
                    nc.tensor.matmul(ps[:, osl], kb[:, KB_FC, :],
                                     coord[:, h * TS:(h + 1) * TS],
                                     start=True, stop=False)
                    nc.tensor.matmul(ps[:, osl], kr[:, KR_MK, :],
                                     xc[:, hsl], start=False, stop=False)
                    nc.tensor.matmul(ps[:, osl], kb[:, KB_M15, :],
                                     p2[:, hsl], start=False, stop=False)
                    nc.tensor.matmul(ps[:, osl], krs[:],
                                     p1[:, hsl], start=False, stop=True)
                    nc.vector.bn_stats(stF[:, q * 2 + h, :], ps[:, osl])
                nc.vector.tensor_copy(fraw[:, sl], ps[:])
            if nxt:                                    # weave P4
                phaseA_compute(nxt, 3)
            c2f = _agg_c2(nc, sm, stF[:], "f")
            pjf = psA.tile([128, 1024], f32, tag="mm")
            nc.tensor.matmul(pjf[:, 0:2], jm[:], c2f[:], start=True, stop=True)
            pjf_s = sm.tile([128, 2], f32)
            nc.vector.tensor_copy(pjf_s[:], pjf[:, 0:2])
            scl3, bia3 = _inorm_scale_bias(nc, sm, pjf_s[:, 0:1], pjf_s[:, 1:2],
                                           pp[:, PP_FUSG:PP_FUSG + 1],
                                           pp[:, PP_FUSB:PP_FUSB + 1], "3")
            fused = big.tile([128, CHS], f32r, tag="fused")
            fsum4 = sm.tile([128, 4], f32)
            for q in range(4):
                sl = slice(q * 1024, (q + 1) * 1024)
                nc.scalar.activation(fused[:, sl], fraw[:, sl], AF.Relu,
                                     bias=bia3[:], scale=scl3[:],
                                     accum_out=fsum4[:, q:q + 1])
            fsum = sm.tile([128, 1], f32)
            nc.vector.tensor_reduce(fsum[:], fsum4[:], axis=AX.X, op=ALU.add)

            # ------------- gate (applied in the expand copies) ------------
            pg = psE.tile([128, 1024], f32, tag="me")
            nc.tensor.matmul(pg[0:1, 0:1], fsum[:], pp[:, PP_GW:PP_GW + 1],
                             start=True, stop=True)
            u = sm.tile([1, 1], f32)
            nc.scalar.activation(u[:], pg[0:1, 0:1], AF.Exp, bias=0.0,
                                 scale=-1.0)
            nc.vector.tensor_scalar_add(u[:], u[:], 1.0)
            nc.vector.reciprocal(u[:], u[:])
            nc.vector.tensor_scalar(u[:], u[:], 1.0 - GATE_FLOOR, GATE_FLOOR,
                                    ALU.mult, ALU.add)
            pgb = psE.tile([128, 1024], f32, tag="me")
            nc.tensor.matmul(pgb[:, 0:1], ones_t[:], u[:], start=True,
                             stop=True)
            gate_bc = sm.tile([128, 1], f32)
            nc.vector.tensor_copy(gate_bc[:], pgb[:, 0:1])
            if debug:
                nc.scalar.dma_start(dbg["d_fused"].ap(), fused[:])
                nc.scalar.dma_start(dbg["d_small"].ap()[:, 4:5], scl3[:])
                nc.scalar.dma_start(dbg["d_small"].ap()[:, 5:6], bia3[:])
                nc.scalar.dma_start(dbg["d_small"].ap()[:, 9:10], gate_bc[:])

            # ------------- expand + store (deferred) -------------
            # Build the 17 expand items for THIS rep; they are emitted at
            # the EW() weave points of the NEXT rep's mid phases (or right
            # here for the final rep).
            assert not exp_items, f"{len(exp_items)} expand items left over"
            exp_items = _make_expand(fused, gate_bc)
            n_inline = len(exp_items) if rep_i == reps - 1 else 0
            for _ in range(n_inline):
                exp_items.pop(0)()
            cur = nxt

    split_multi_waits(nc)
    return nc


def _get_program():
    if "nc" not in _PROG_CACHE:
        _PROG_CACHE["nc"] = build_program()
    return _PROG_CACHE["nc"]


_PROG_CACHE = {}


def make_in_maps(inputs):
    """Host-side preprocessing: full inputs dict -> per-core in_maps."""
    gi = {k: np.asarray(v, dtype=np.float32) for k, v in inputs.items()}

    x = np.ascontiguousarray(gi["x"].reshape(NCORES, C_IN, S))
    x_bf = x.astype(ml_dtypes.bfloat16)

    Wc = gi["compress_w"]                       # [32, 512]
    K_ = _block_diag(gi["cot_key_w"])           # [32, 32]
    V_ = _block_diag(gi["cot_val_w"])
    A1 = np.zeros((M, 2 * M), np.float32)
    a1 = gi["cot_att1_w"]                       # [4, 8, 16]
    for g in range(G):
        A1[g * MG:(g + 1) * MG, g * 2 * MG:(g + 1) * 2 * MG] = a1[g]
    A1eff = A1[:, :M] @ K_ + A1[:, M:]
    A2 = _block_diag(gi["cot_att2_w"])
    CF = gi["cot_fuse_w"]                       # [32, 32]
    F = gi["fusion_w"]                          # [32, 64]
    Fc, Fct = F[:, :M], F[:, M:]
    M_ = Fct @ CF
    MK = M_ @ K_
    E = gi["expand_w"]                          # [512, 32]

    # compress lhsT [128, 4, 32]: cw[p, kc, c] = Wc[c, kc*128+p]
    WcT = np.ascontiguousarray(Wc.T)            # [512, 32]
    cw = np.zeros((128, 4, 32), np.float32)
    for kc in range(4):
        cw[:, kc, :] = WcT[kc * 128:(kc + 1) * 128, :]
    cw = cw.astype(ml_dtypes.bfloat16)
    # expand lhsT: E.T replicated per chunk block (row tile_position)
    ewt = np.tile(np.ascontiguousarray(E.T), (NCH, 1))   # [128, 512]
    kr = np.zeros((128, 5, 128), np.float32)
    for i, mat in enumerate([A1eff, A2, V_, MK, M_]):
        kr[:, i, :] = _kron128(mat)
    kb = np.zeros((128, 2, 128), np.float32)
    for i, mat in enumerate([Fc, (1.0 - COT_LAM) / 2.0 * M_]):
        kb[:, i, :] = _kron128(mat)
    kb = kb.astype(ml_dtypes.bfloat16)
    jm = np.kron(np.ones((NCH, NCH), np.float32), np.eye(M, dtype=np.float32))
    cm = np.ascontiguousarray(_block_diag(gi["coord_proj_w"]).T)   # [32, 32]
    # pz lhsT [32, 2, 128]: out partitions = 4 chunk replicas of 32 chans
    c2m = np.zeros((32, 2, 128), np.float32)
    c2m[:, 0, :] = np.tile(_block_diag(gi["coord_h_w"]).T, (1, NCH))
    c2m[:, 1, :] = np.tile(_block_diag(gi["coord_wc_w"]).T, (1, NCH))

    pp = np.zeros((128, 8), np.float32)
    rep = lambda v: np.tile(np.asarray(v, np.float32), NCH)
    pp[:, PP_INCG] = rep(gi["inc_g"])
    pp[:, PP_INCB] = rep(gi["inc_b"])
    pp[:, PP_ATTG] = rep(gi["cot_attn_g"])
    pp[:, PP_ATTB] = rep(gi["cot_attn_b"])
    pp[:, PP_FUSG] = rep(gi["fus_g"])
    pp[:, PP_FUSB] = rep(gi["fus_b"])
    pp[:, PP_GW] = rep(gi["gate_w"].reshape(-1) / float(S))

    # coord inorm gamma/beta stacked [h|w] (same params for both halves)
    cp = np.zeros((32, 4), np.float32)
    cp[:, 0] = gi["coord_proj_g"]
    cp[:, 1] = gi["coord_proj_g"]
    cp[:, 2] = gi["coord_proj_b"]
    cp[:, 3] = gi["coord_proj_b"]
    # mean-rescale constants: [alpha*H | beta*W] per partition
    cc = np.zeros((128, 2), np.float32)
    cc[:, 0] = float(gi["coord_alpha"]) * H
    cc[:, 1] = float(gi["coord_beta"]) * W

    on = np.ones((1, 128), np.float32)
    shared = dict(cw=cw, ew=ewt, kr=kr, kb=kb, jm=jm, cm=cm, c2m=c2m,
                  pp=pp, cp=cp, cc=cc, on=on)
    return [dict(shared, x=np.ascontiguousarray(x_bf[i]))
            for i in range(NCORES)]


def kernel(**inputs):
    nc = _get_program()
    in_maps = make_in_maps(inputs)
    res = run_bass_kernel_spmd(nc, in_maps, list(range(NCORES)))
    out = np.stack([np.asarray(res.results[i]["y"])
                    .astype(np.float32).reshape(C_IN, H, W)
                    for i in range(NCORES)], axis=0)
    return out



# revision 26
# speedup vs baseline: 1.4988x; 1.4988x over previous
"""Trainium2 Bass kernel for nn_ChannelSpatialContextAttention.

Sharding: pure data-parallel - batch B=8, one image per NeuronCore.

Per core (batch dim dropped): x [512, 16384] -> y [512, 16384].
All grouped 1x1 convs are tiny channel mixes; block-diagonalized and
algebraically fused on the host:

    xc      = relu(inorm(Wc @ x))                       compress 512->32
    att_pre = A1eff @ xc        (A1eff = A1[:, :32]@K + A1[:, 32:])
    att     = relu(inorm(att_pre))
    logits  = A2 @ att
    sm*lam  = exp(logits/tau) * (lam * S / S_c)         max-free softmax
    amv     = sm*lam*v + (1-lam)*sigmoid(logits)*v      v = V @ xc
    coord   = (alpha*ah(c,h) + beta*aw(c,w)) * xc       coord attention
    fused   = relu(inorm(Fc@coord + (M@K)@xc + 0.15M@p2 + Msm@p1))
    gate    = sigmoid(gw . mean(fused)) * 0.95 + 0.05
    y       = E @ fused * gate                          expand 32->512

Layout: S-sized tensors in SBUF as [128, 4096], partition p = chunk*32+c
(4 spatial chunks of 4096).  IO is bf16 both ways (host converts): halves
HBM traffic; all big matmuls use bf16 lhsT x bf16 rhs (walrus forbids
mixing 32-bit with 16-bit operands) at 1 cyc/row.  Compress packs 4
chunk-blocks into full [128,1024] PSUM tiles via PSUM column offsets so
stats/copies run on 128 partitions.  Cross-chunk sums -> J matmul.

Pipeline structure (2-deep software pipeline; reps overlap):
- input DMAs on SP queue, ring-split patch pins SP DMAs to HWDGE rings
  0-3 and ACT DMAs (outputs + small) to 4-7 so ring completion counters
  don't chain rep k+1's inputs behind rep k's outputs.
- phase A (input DMA + compress) of rep k+1 is woven into rep k's mid
  phases at P0..P4; P0 sits at the rep top so the first input DMA
  anchors right after the previous rep's gate.
- the EXPAND of rep k is deferred: its 16 (tg,mc) groups + dma flush
  are emitted at ~19 EW() weave points inside rep k+1's mid phases, so
  the output stream + PSUM evacuation overlap the next rep's compute
  instead of serializing at the rep tail.  One yb = [128,4096] bf16 ->
  one 1MiB DMA, issued one group LATE so its sem waits are satisfied
  (no ACT.SEQ head-of-line stall).  Gated copies split ACT:DVE 38:26
  to time-balance both engines.
- barrier chains (inorm scale/bias via J-matmul cross-chunk combine,
  softmax esum, gate) run at scheduler priority 0 so their tiny ops
  never queue behind woven bulk work; chains use 1 Newton rsqrt
  iteration (2 for the output-scaling fused inorm), host-negated
  gammas, 0.25-scaled jm2, and lam*S folded into KR_M.
- the coord branch runs pair-vectorized (h and w stacked [*, 2, 128]) on
  replicated 128-partition tiles (host-tiled lhsT), softmax without max
  subtraction; wlin on the idle Pool engine.
"""

import os
import numpy as np
import ml_dtypes

import concourse.bass as bass
import concourse.tile as tile
import concourse.mybir as mybir
from concourse.bass_utils import run_bass_kernel_spmd

try:  # persistent NEFF compile cache across calls/processes (best effort)
    import jax
    jax.config.update("jax_compilation_cache_dir", "/tmp/jax_cc_cache")
    jax.config.update("jax_persistent_cache_min_compile_time_secs", 0)
except Exception:
    pass

dt = mybir.dt
AF = mybir.ActivationFunctionType
ALU = mybir.AluOpType
AX = mybir.AxisListType

NCORES = 8
C_IN = 512
M = 32
G = 4
MG = M // G
H = 128
W = 128
S = H * W
TS = 512
NCH = 4
CHS = S // NCH
EPS = 1e-5
COT_TAU = 0.8
COT_LAM = 0.7
GATE_FLOOR = 0.05

f32 = dt.float32
f32r = dt.float32r
bf16 = dt.bfloat16

KR_A1EFF, KR_A2, KR_V, KR_MK, KR_M = range(5)
KB_FC, KB_M15 = range(2)
PP_INCG, PP_INCB, PP_ATTG, PP_ATTB, PP_FUSG, PP_FUSB, PP_GW = range(7)

_RING_PATCHED = False


def _patch_ring_split():
    """Pin SP-issued HWDGE DMAs to rings 0-3 and other engines' to 4-7.

    The stock round-robin mixes input and output DMAs on the same rings;
    ring completion sems are counters, so a WAW dep on an input DMA's
    ring tick transitively waits for every earlier DMA on that ring --
    including the previous rep's output stream. Splitting the rings
    breaks that cross-coupling.
    """
    global _RING_PATCHED
    if _RING_PATCHED:
        return
    _RING_PATCHED = True
    import concourse.tile_sem_assignment as tsa
    from concourse.tile_scheduler import DMAInst
    from concourse import bass_isa

    orig = tsa.TileClockTick._assign_tick

    def _assign_tick_split(self, inst):
        if (isinstance(inst, DMAInst)
                and not isinstance(inst, bass_isa.UserSyncedRemoteDMADescs)
                and inst.engine != mybir.EngineType.Pool):
            cnts = getattr(self, "_ring_cnts", None)
            if cnts is None:
                cnts = self._ring_cnts = {}
            grp = 0 if inst.engine == mybir.EngineType.SP else 1
            k = cnts.get(grp, 0)
            cnts[grp] = k + 1
            self.next_hw_dma_idx = grp * 4 + (k % 4)
        return orig(self, inst)

    tsa.TileClockTick._assign_tick = _assign_tick_split


def _block_diag(w):
    g, o, i = w.shape
    out = np.zeros((g * o, g * i), np.float32)
    for k in range(g):
        out[k * o:(k + 1) * o, k * i:(k + 1) * i] = w[k]
    return out


def _kron128(w):
    """W [32,32] (out,in) -> lhsT [128,128] = kron(I4, W.T)."""
    return np.kron(np.eye(NCH, dtype=np.float32),
                   np.ascontiguousarray(w.T, dtype=np.float32))


def _newton_rsqrt(eng, pool, y_out, v_in, shape, tagp, iters=2):
    """y_out sbuf fp32 = rsqrt(v_in) elementwise; no ACT table needed."""
    yi = pool.tile(shape, dt.int32, tag="nri" + tagp)
    eng.tensor_single_scalar(yi[:], v_in.bitcast(dt.int32), 1,
                             ALU.logical_shift_right)
    eng.tensor_scalar(yi[:], yi[:], -1, 0x5F3759DF, ALU.mult, ALU.add)
    y = yi[:].bitcast(f32)
    half = pool.tile(shape, f32, tag="nrh" + tagp)
    eng.tensor_scalar_mul(half[:], v_in, 0.5)
    t = pool.tile(shape, f32, tag="nrt" + tagp)
    for _ in range(iters):
        eng.tensor_tensor(t[:], y, y, ALU.mult)
        eng.tensor_tensor(t[:], half[:], t[:], ALU.mult)
        eng.tensor_scalar(t[:], t[:], -1.0, 1.5, ALU.mult, ALU.add)
        eng.tensor_tensor(y, y, t[:], ALU.mult)
    eng.tensor_copy(y_out, y)


def _inorm_scale_bias(nc, pool, pj_mean, pj_e2, g_ap, b_ap, tagp):
    """From J-combined [mean, E2] ([128,1] sbuf aps): returns (scale, bias)
    [128,1] sbuf aps for relu(x*scale+bias)."""
    eng = nc.vector
    var = pool.tile([128, 1], f32, tag="inv" + tagp)
    eng.tensor_tensor(var[:], pj_mean, pj_mean, ALU.mult)
    eng.tensor_tensor(var[:], pj_e2, var[:], ALU.subtract)
    eng.tensor_scalar_add(var[:], var[:], EPS)
    rs = pool.tile([128, 1], f32, tag="inr" + tagp)
    _newton_rsqrt(eng, pool, rs[:], var[:], [128, 1], tagp)
    scl = pool.tile([128, 1], f32, tag="ins" + tagp)
    eng.tensor_tensor(scl[:], rs[:], g_ap, ALU.mult)
    nscl = pool.tile([128, 1], f32, tag="inn" + tagp)
    eng.tensor_scalar_mul(nscl[:], scl[:], -1.0)
    bia = pool.tile([128, 1], f32, tag="inb" + tagp)
    eng.tensor_tensor(bia[:], pj_mean, nscl[:], ALU.mult)
    eng.tensor_tensor(bia[:], bia[:], b_ap, ALU.add)
    return scl, bia


def split_multi_waits(nc):
    """This env's walrus supports at most one sync-wait per instruction:
    hoist extra waits onto same-engine NOPs inserted just before."""
    for f in nc.m.functions:
        for bb in f.blocks:
            il = bb.instructions
            out = []
            dirty = False
            for ins in il:
                si = ins.sync_info
                waits = list(si.on_wait) if si is not None else []
                if len(waits) > 1:
                    dirty = True
                    for k, w in enumerate(waits[:-1]):
                        nop = mybir.InstNoOp(
                            name=f"wsplit_{ins.name}_{k}", ins=[], outs=[])
                        nop.engine = ins.engine
                        nop.sync_info = mybir.SyncInfo(on_wait=[w],
                                                       on_update=[])
                        out.append(nop)
                    ins.sync_info = mybir.SyncInfo(
                        on_wait=[waits[-1]], on_update=list(si.on_update))
                out.append(ins)
            if dirty:
                bb.instructions = out


def _agg_c2(nc, pool, st, tagp):
    """bn_stats buffer [128, n, 6] -> c2 [128,2] = [mean/4, (var+mean^2)/4]
    (scaled so a J-sum over the 4 chunks yields full-channel mean/E2)."""
    ag = pool.tile([128, 2], f32, tag="ag" + tagp)
    nc.vector.bn_aggr(ag[:], st[:])
    c2 = pool.tile([128, 2], f32, tag="c2" + tagp)
    nc.vector.tensor_scalar_mul(c2[:, 0:1], ag[:, 0:1], 0.25)
    e2 = pool.tile([128, 1], f32, tag="e2" + tagp)
    nc.vector.scalar_tensor_tensor(e2[:], ag[:, 0:1], ag[:, 0:1], ag[:, 1:2],
                                   ALU.mult, ALU.add)
    nc.vector.tensor_scalar_mul(c2[:, 1:2], e2[:], 0.25)
    return c2


def build_program(debug=False, reps=1):
    _patch_ring_split()
    nc = bass.Bass("TRN2", target_bir_lowering=False, debug=False,
                   num_devices=NCORES)

    x_e = nc.dram_tensor("x", [C_IN, S], bf16, kind="ExternalInput")
    cw_e = nc.dram_tensor("cw", [128, 4, 32], bf16, kind="ExternalInput")
    ew_e = nc.dram_tensor("ew", [128, 512], f32, kind="ExternalInput")
    kr_e = nc.dram_tensor("kr", [128, 5, 128], f32, kind="ExternalInput")
    kb_e = nc.dram_tensor("kb", [128, 2, 128], bf16, kind="ExternalInput")
    jm_e = nc.dram_tensor("jm", [128, 128], f32, kind="ExternalInput")
    cm_e = nc.dram_tensor("cm", [32, 32], f32, kind="ExternalInput")
    c2_e = nc.dram_tensor("c2m", [32, 2, 128], f32, kind="ExternalInput")
    pp_e = nc.dram_tensor("pp", [128, 8], f32, kind="ExternalInput")
    cp_e = nc.dram_tensor("cp", [32, 4], f32, kind="ExternalInput")
    cc_e = nc.dram_tensor("cc", [128, 2], f32, kind="ExternalInput")
    on_e = nc.dram_tensor("on", [1, 128], f32, kind="ExternalInput")
    y_e = nc.dram_tensor("y", [C_IN, S], bf16, kind="ExternalOutput")
    dbg = {}
    if debug:
        for nm in ["d_xc", "d_coord", "d_p1", "d_p2", "d_fused"]:
            dbg[nm] = nc.dram_tensor(nm, [128, CHS], bf16,
                                     kind="ExternalOutput")
        dbg["d_small"] = nc.dram_tensor("d_small", [128, 16], f32,
                                        kind="ExternalOutput")
        dbg["d_ahw"] = nc.dram_tensor("d_ahw", [128, 2, 128], bf16,
                                      kind="ExternalOutput")

    with tile.TileContext(nc) as tc:
      with tc.tile_pool(name="wpool", bufs=1) as wp, \
           tc.tile_pool(name="stream", bufs=5) as strm, \
           tc.tile_pool(name="big", bufs=1) as big, \
           tc.tile_pool(name="xcr", bufs=2) as xcr, \
           tc.tile_pool(name="afp", bufs=1) as afp, \
           tc.tile_pool(name="thp", bufs=2) as thp, \
           tc.tile_pool(name="wlinp", bufs=2) as wlp, \
           tc.tile_pool(name="crdp", bufs=3) as crdp, \
           tc.tile_pool(name="ybuf", bufs=3) as ybp, \
           tc.tile_pool(name="small", bufs=1) as sm, \
           tc.tile_pool(name="psA", bufs=2, space="PSUM") as psA, \
           tc.tile_pool(name="psE", bufs=2, space="PSUM") as psE:
        # ------------- weights / params (loaded once, SP queue) ------
        cw = wp.tile([128, 4, 32], bf16, tag="cw")
        nc.sync.dma_start(cw[:], cw_e.ap())
        ew = wp.tile([128, 512], f32r, tag="ew")
        nc.sync.dma_start(ew[:], ew_e.ap().bitcast(f32r))
        kr = wp.tile([128, 5, 128], f32r, tag="kr")
        nc.sync.dma_start(kr[:], kr_e.ap().bitcast(f32r))
        kb = wp.tile([128, 2, 128], bf16, tag="kb")
        nc.sync.dma_start(kb[:], kb_e.ap())
        jm = wp.tile([128, 128], f32, tag="jm")
        nc.sync.dma_start(jm[:], jm_e.ap())
        cm = wp.tile([32, 32], f32r, tag="cm")
        nc.sync.dma_start(cm[:], cm_e.ap().bitcast(f32r))
        cm2 = wp.tile([32, 2, 128], f32r, tag="cm2")
        nc.sync.dma_start(cm2[:], c2_e.ap().bitcast(f32r))
        pp = wp.tile([128, 8], f32, tag="pp")
        nc.sync.dma_start(pp[:], pp_e.ap())
        cp = wp.tile([32, 4], f32, tag="cp")
        nc.sync.dma_start(cp[:], cp_e.ap())
        cc = wp.tile([128, 2], f32, tag="cc")
        nc.sync.dma_start(cc[:], cc_e.ap())
        ones_t = wp.tile([1, 128], f32, tag="ones_t")
        nc.sync.dma_start(ones_t[:], on_e.ap())

        x_r = x_e.ap().rearrange("(kc p) s -> p kc s", p=128)

        # ---- software-pipelined phase A (compress) ----
        # rep k+1's input DMAs + compress quarters are woven between rep
        # k's phases (DMA for quarter j issues one weave point before its
        # compress) so the input stream spreads across the whole rep and
        # compress matmuls fill PE barrier stalls.
        def phaseA_alloc():
            return {"xcraw": xcr.tile([128, CHS], f32, tag="xcraw",
                                      name="xcraw"),
                    "stA": sm.tile([128, 8, 6], f32, tag="stA",
                                   name="stA"),
                    "xins": {}}

        def phaseA_dma(st, j):
            xins = []
            for ch in range(NCH):
                xin = strm.tile([128, 4, 1024], bf16, tag="xin")
                nc.sync.dma_start(
                    xin[:],
                    x_r[:, :, ch * CHS + j * 1024:
                        ch * CHS + (j + 1) * 1024])
                xins.append(xin)
            st["xins"][j] = xins

        def phaseA_compute(st, j):
            xins = st["xins"].pop(j)
            ps = psA.tile([128, 1024], f32, tag="mm")
            for ch in range(NCH):
                for kc in range(4):
                    for h in range(2):
                        nc.tensor.matmul(
                            ps[ch * 32:(ch + 1) * 32,
                               h * TS:(h + 1) * TS],
                            cw[:, kc, :],
                            xins[ch][:, kc, h * TS:(h + 1) * TS],
                            start=(kc == 0), stop=(kc == 3),
                            tile_position=(0, ch * 32))
            for h in range(2):
                nc.vector.bn_stats(st["stA"][:, j * 2 + h, :],
                                   ps[:, h * TS:(h + 1) * TS])
            nc.scalar.copy(st["xcraw"][:, j * 1024:(j + 1) * 1024], ps[:])

        # ---- expand phase of rep k, emitted as 17 deferred items (16
        # (tg,mc) groups + dma flush) woven into rep k+1's mid phases so
        # the output stream + PSUM evacuation overlap the next rep's
        # compute instead of serializing at the rep tail. ----
        def _make_expand(fused_t, gate_t):
            st = {"pending": None, "ei": 0}

            def mk(tg, mc):
                def g():
                    yb = ybp.tile([128, 4096], bf16, tag="yb", name="yb")
                    for k in range(4):
                        ps2 = psE.tile([128, 1024], f32, tag="me",
                                       name="ps2")
                        for h in range(2):
                            nc.tensor.matmul(
                                ps2[:, h * TS:(h + 1) * TS],
                                ew[tg * 32:(tg + 1) * 32,
                                   mc * 128:(mc + 1) * 128],
                                fused_t[tg * 32:(tg + 1) * 32,
                                        k * 1024 + h * TS:
                                        k * 1024 + (h + 1) * TS],
                                start=True, stop=True,
                                tile_position=(tg * 32, 0))
                        dstp = yb[:, k * 1024:(k + 1) * 1024]
                        ei = st["ei"]
                        st["ei"] = ei + 1
                        # 23/64 of the gated copies go to DVE (time-balances
                        # ACT@1.038us vs DVE@1.192us per [128,1024] copy)
                        if (ei + 1) * 23 // 64 > ei * 23 // 64:
                            nc.vector.tensor_scalar_mul(dstp, ps2[:],
                                                        gate_t[:])
                        else:
                            nc.scalar.mul(dstp, ps2[:], gate_t[:])
                        if k == 0 and st["pending"] is not None:
                            nc.scalar.dma_start(*st["pending"])
                            st["pending"] = None
                    st["pending"] = (y_e.ap()[mc * 128:(mc + 1) * 128,
                                              tg * 4096:(tg + 1) * 4096],
                                     yb[:])
                return g

            items = [mk(tg, mc) for tg in range(4) for mc in range(4)]

            def flush():
                if st["pending"] is not None:
                    nc.scalar.dma_start(*st["pending"])
                    st["pending"] = None

            items.append(flush)
            return items

        exp_items = []

        def EW():
            if exp_items:
                exp_items.pop(0)()

        # prologue: rep 0's phase A runs un-pipelined
        cur = phaseA_alloc()
        for j in range(4):
            phaseA_dma(cur, j)
            phaseA_compute(cur, j)

        for rep_i in range(reps):
            nxt = None
            xcraw, stA = cur["xcraw"], cur["stA"]
            # ------------- xc inorm + relu -------------
            c2a = _agg_c2(nc, sm, stA[:], "a")
            pja = psA.tile([128, 1024], f32, tag="mm")
            nc.tensor.matmul(pja[:, 0:2], jm[:], c2a[:], start=True, stop=True)
            pja_s = sm.tile([128, 2], f32)
            nc.vector.tensor_copy(pja_s[:], pja[:, 0:2])
            scl1, bia1 = _inorm_scale_bias(nc, sm, pja_s[:, 0:1], pja_s[:, 1:2],
                                           pp[:, PP_INCG:PP_INCG + 1],
                                           pp[:, PP_INCB:PP_INCB + 1], "1")
            if rep_i + 1 < reps:                       # weave P0
                nxt = phaseA_alloc()
                phaseA_dma(nxt, 0)
            # xc relu + early coord reductions + attpre, interleaved per
            # chunk so the coord latency chain starts ASAP while the PE
            # stream runs attpre with no coord tinies in front of it.
            xc = big.tile([128, CHS], f32r, tag="xc")
            xcf = xc[:].bitcast(f32)
            zhp = sm.tile([128, 32], f32)
            zwp4 = sm.tile([128, 4, W], f32)
            attpre = afp.tile([128, CHS], f32, tag="af")
            stB = sm.tile([128, 8, 6], f32)
            for q in range(4):
                sl = slice(q * 1024, (q + 1) * 1024)
                nc.scalar.activation(xc[:, sl], xcraw[:, sl], AF.Relu,
                                     bias=bia1[:], scale=scl1[:])
                nc.vector.tensor_reduce(
                    zhp[:, q * 8:(q + 1) * 8],
                    xcf[:, sl].rearrange("p (a b) -> p a b", b=W),
                    axis=AX.X, op=ALU.add)
                nc.vector.tensor_reduce(
                    zwp4[:, q, :],
                    xcf[:, sl].rearrange("p (a b) -> p b a", b=W),
                    axis=AX.X, op=ALU.add)
                ps = psE.tile([128, 1024], f32, tag="me")
                for h in range(2):
                    nc.tensor.matmul(ps[:, h * TS:(h + 1) * TS],
                                     kr[:, KR_A1EFF, :],
                                     xc[:, q * 1024 + h * TS:
                                        q * 1024 + (h + 1) * TS],
                                     start=True, stop=True)
                    nc.vector.bn_stats(stB[:, q * 2 + h, :],
                                       ps[:, h * TS:(h + 1) * TS])
                nc.vector.tensor_copy(attpre[:, sl], ps[:])
            if nxt:                                    # weave P1
                phaseA_compute(nxt, 0)
                phaseA_dma(nxt, 1)
            # cross-chunk combine for zw (DVE) + zh assembly into zc
            zwp = sm.tile([128, W], f32)
            nc.vector.tensor_reduce(
                zwp[:], zwp4[:].rearrange("p q w -> p w q"),
                axis=AX.X, op=ALU.add)
            zhps = sm.tile([128, 32], f32)
            nc.vector.tensor_scalar_mul(zhps[:], zhp[:], 1.0 / W)
            zc = sm.tile([32, 2, 128], f32r)
            for ch in range(NCH):
                nc.scalar.dma_start(zc[:, 0, ch * 32:(ch + 1) * 32],
                                    zhps[ch * 32:(ch + 1) * 32, :]
                                    .bitcast(f32r))

            # ------------- att inorm + relu -------------
            c2b = _agg_c2(nc, sm, stB[:], "b")
            pjb = psA.tile([128, 1024], f32, tag="mm")
            nc.tensor.matmul(pjb[:, 0:2], jm[:], c2b[:], start=True, stop=True)
            pjb_s = sm.tile([128, 2], f32)
            nc.vector.tensor_copy(pjb_s[:], pjb[:, 0:2])
            scl2, bia2 = _inorm_scale_bias(nc, sm, pjb_s[:, 0:1], pjb_s[:, 1:2],
                                           pp[:, PP_ATTG:PP_ATTG + 1],
                                           pp[:, PP_ATTB:PP_ATTB + 1], "2")
            # ------- logits + v + tanh/exp + p1/p2 per 1024-chunk -------
            # tanh/exp read the logits PSUM tile directly (no SBUF staging).
            # p1 = exp_t * v (softmax scale folded into krs lhsT)
            # p2 = (tanh(logits/2) + 1) * v (0.15 folded into M15 lhsT)
            # max-free softmax: post-inorm logits are bounded well inside
            # fp32 exp range, so no max subtraction barrier is needed.
            esum4 = sm.tile([128, 4], f32)
            p1 = big.tile([128, CHS], bf16, tag="p1")
            p2 = big.tile([128, CHS], bf16, tag="p2")
            for q in range(4):
                sl = slice(q * 1024, (q + 1) * 1024)
                atq = thp.tile([128, 1024], f32r, tag="att")
                nc.scalar.activation(atq[:], attpre[:, sl], AF.Relu,
                                     bias=bia2[:], scale=scl2[:])
                lps = psA.tile([128, 1024], f32, tag="mm")
                for h in range(2):
                    nc.tensor.matmul(lps[:, h * TS:(h + 1) * TS],
                                     kr[:, KR_A2, :],
                                     atq[:, h * TS:(h + 1) * TS],
                                     start=True, stop=True)
                psv = psE.tile([128, 1024], f32, tag="me")
                for h in range(2):
                    nc.tensor.matmul(psv[:, h * TS:(h + 1) * TS],
                                     kr[:, KR_V, :],
                                     xc[:, q * 1024 + h * TS:
                                        q * 1024 + (h + 1) * TS],
                                     start=True, stop=True)
                th_t = thp.tile([128, 1024], bf16, tag="th")
                exp_t = thp.tile([128, 1024], bf16, tag="exp")
                nc.scalar.activation(th_t[:], lps[:], AF.Tanh,
                                     bias=0.0, scale=0.5)
                nc.scalar.activation(exp_t[:], lps[:], AF.Exp,
                                     bias=0.0, scale=1.0 / COT_TAU,
                                     accum_out=esum4[:, q:q + 1])
                nc.vector.tensor_tensor(p1[:, sl], exp_t[:], psv[:],
                                        ALU.mult)
                nc.vector.scalar_tensor_tensor(p2[:, sl], th_t[:],
                                               1.0, psv[:],
                                               ALU.add, ALU.mult)

            if nxt:                                    # weave P2
                phaseA_compute(nxt, 1)
                phaseA_dma(nxt, 2)
            esumS = sm.tile([128, 1], f32)
            nc.vector.tensor_reduce(esumS[:], esum4[:], axis=AX.X, op=ALU.add)
            pS = psA.tile([128, 1024], f32, tag="mm")
            nc.tensor.matmul(pS[:, 0:1], jm[:], esumS[:], start=True,
                             stop=True)
            recS = sm.tile([128, 1], f32)
            nc.vector.reciprocal(recS[:], pS[:, 0:1])
            smscl = sm.tile([128, 1], f32)
            nc.vector.tensor_scalar_mul(smscl[:], recS[:], COT_LAM * S)
            # krs = M_kron * smscl (per-contraction-row scale)
            krs = wp.tile([128, 128], bf16, tag="krs")
            nc.vector.tensor_scalar_mul(krs[:], kr[:, KR_M, :].bitcast(f32),
                                        smscl[:])
            if debug:
                nc.scalar.dma_start(dbg["d_p1"].ap(), p1[:])
                nc.scalar.dma_start(dbg["d_p2"].ap(), p2[:])
                nc.scalar.dma_start(dbg["d_small"].ap()[:, 7:8], smscl[:])

            # ---- coord tinies, pair-vectorized (h|w stacked on free dim),
            # late in PE order so they never stall the main PE stream ----
            pzw = psE.tile([128, 1024], f32, tag="me")
            nc.tensor.matmul(pzw[:, 0:W], jm[:], zwp[:], start=True, stop=True)
            nc.vector.tensor_scalar_mul(zc[:, 1, :], pzw[0:32, 0:W], 1.0 / H)
            # proj = silu(inorm(P @ [zh|zw]))
            ppj = psE.tile([128, 1024], f32, tag="me")
            nc.tensor.matmul(ppj[0:32, 0:256],
                             cm[:], zc[:].rearrange("p a b -> p (a b)"),
                             start=True, stop=True)
            ppj_s = sm.tile([32, 2, 128], f32)
            nc.vector.tensor_copy(ppj_s[:], ppj[0:32, 0:256]
                                  .rearrange("p (a b) -> p a b", b=128))
            stp = sm.tile([32, 2, 6], f32)
            agp = sm.tile([32, 2, 2], f32)
            for k in range(2):
                nc.vector.bn_stats(stp[:, k, :], ppj_s[:, k, :])
                nc.vector.bn_aggr(agp[:, k, :], stp[:, k, :])
            vp = sm.tile([32, 2], f32)
            nc.vector.tensor_scalar_add(vp[:], agp[:, :, 1], EPS)
            rsp = sm.tile([32, 2], f32)
            _newton_rsqrt(nc.vector, sm, rsp[:], vp[:], [32, 2], "c")
            sclp = sm.tile([32, 2], f32)
            nc.vector.tensor_tensor(sclp[:], rsp[:], cp[:, 0:2], ALU.mult)
            nsclp = sm.tile([32, 2], f32)
            nc.vector.tensor_scalar_mul(nsclp[:], sclp[:], -1.0)
            biap = sm.tile([32, 2], f32)
            nc.vector.tensor_tensor(biap[:], agp[:, :, 0], nsclp[:], ALU.mult)
            nc.vector.tensor_tensor(biap[:], biap[:], cp[:, 2:4], ALU.add)
            ut = sm.tile([32, 2, 128], f32)
            nc.vector.tensor_tensor(ut[:], ppj_s[:],
                                    sclp[:].unsqueeze(2)
                                    .broadcast_to([32, 2, 128]), ALU.mult)
            nc.vector.tensor_tensor(ut[:], ut[:],
                                    biap[:].unsqueeze(2)
                                    .broadcast_to([32, 2, 128]), ALU.add)
            # silu(u) = u * (0.5 + 0.5*tanh(u/2))
            sg = sm.tile([32, 2, 128], f32)
            nc.scalar.activation(sg[:], ut[:], AF.Tanh, bias=0.0, scale=0.5)
            nc.vector.tensor_scalar(sg[:], sg[:], 0.5, 0.5, ALU.mult, ALU.add)
            proj = sm.tile([32, 2, 128], f32r)
            nc.vector.tensor_tensor(proj[:], sg[:], ut[:], ALU.mult)
            # z_h/z_w projections, replicated out to 128 partitions via
            # host-tiled lhsT; then pairwise mean-rescaled softmax, no max
            # subtraction (post-inorm values are tiny).
            pzf = psE.tile([128, 1024], f32, tag="me")
            for k in range(2):
                nc.tensor.matmul(pzf[:, k * 128:(k + 1) * 128],
                                 cm2[:, k, :], proj[:, k, :],
                                 start=True, stop=True)
            ex = sm.tile([128, 2, 128], f32)
            nc.scalar.activation(ex[:], pzf[:, 0:256]
                                 .rearrange("p (a b) -> p a b", b=128),
                                 AF.Exp, bias=0.0, scale=1.0)
            es = sm.tile([128, 2], f32)
            nc.vector.tensor_reduce(es[:], ex[:], axis=AX.X, op=ALU.add)
            escl = sm.tile([128, 2], f32)
            nc.vector.reciprocal(escl[:], es[:])
            nc.vector.tensor_tensor(escl[:], escl[:], cc[:], ALU.mult)
            a_s = sm.tile([128, 2, 128], bf16)
            nc.vector.tensor_tensor(a_s[:], ex[:],
                                    escl[:].unsqueeze(2)
                                    .broadcast_to([128, 2, 128]), ALU.mult)
            ah128b = sm.tile([128, 32], bf16)
            for ch in range(NCH):
                nc.vector.tensor_copy(
                    ah128b[ch * 32:(ch + 1) * 32, :],
                    a_s[ch * 32:(ch + 1) * 32, 0,
                        ch * 32:(ch + 1) * 32])
            if nxt:                                    # weave P3
                phaseA_compute(nxt, 2)
                phaseA_dma(nxt, 3)

            # ------------- fused (wlin/coord built per quarter) -----------
            fraw = afp.tile([128, CHS], f32, tag="af")
            stF = sm.tile([128, 8, 6], f32)
            for q in range(4):
                sl = slice(q * 1024, (q + 1) * 1024)
                wlin = wlp.tile([128, 8, W], bf16, tag="wlin")
                nc.gpsimd.tensor_tensor(
                    wlin[:],
                    ah128b[:, q * 8:(q + 1) * 8].unsqueeze(2)
                    .broadcast_to([128, 8, W]),
                    a_s[:, 1, :].unsqueeze(1).broadcast_to([128, 8, W]),
                    ALU.add)
                coord = crdp.tile([128, 1024], bf16, tag="coord")
                nc.gpsimd.tensor_tensor(coord[:],
                                        wlin[:].rearrange("p a b -> p (a b)"),
                                        xcf[:, sl], ALU.mult)
                ps = psA.tile([128, 1024], f32, tag="mm")
                for h in range(2):
                    hsl = slice(q * 1024 + h * TS, q * 1024 + (h + 1) * TS)
                    osl = slice(h * TS, (h + 1) * TS)
                    nc.tensor.matmul(ps[:, osl], kb[:, KB_FC, :],
                                     coord[:, h * TS:(h + 1) * TS],
                                     start=True, stop=False)
                    nc.tensor.matmul(ps[:, osl], kr[:, KR_MK, :],
                                     xc[:, hsl], start=False, stop=False)
                    nc.tensor.matmul(ps[:, osl], kb[:, KB_M15, :],
                                     p2[:, hsl], start=False, stop=False)
                    nc.tensor.matmul(ps[:, osl], krs[:],
                                     p1[:, hsl], start=False, stop=True)
                    nc.vector.bn_stats(stF[:, q * 2 + h, :], ps[:, osl])
                nc.vector.tensor_copy(fraw[:, sl], ps[:])
            if nxt:                                    # weave P4
                phaseA_compute(nxt, 3)
            c2f = _agg_c2(nc, sm, stF[:], "f")
            pjf = psA.tile([128, 1024], f32, tag="mm")
            nc.tensor.matmul(pjf[:, 0:2], jm[:], c2f[:], start=True, stop=True)
            pjf_s = sm.tile([128, 2], f32)
            nc.vector.tensor_copy(pjf_s[:], pjf[:, 0:2])
            scl3, bia3 = _inorm_scale_bias(nc, sm, pjf_s[:, 0:1], pjf_s[:, 1:2],
                                           pp[:, PP_FUSG:PP_FUSG + 1],
                                           pp[:, PP_FUSB:PP_FUSB + 1], "3")
            fused = big.tile([128, CHS], f32r, tag="fused")
            fsum4 = sm.tile([128, 4], f32)
            for q in range(4):
                sl = slice(q * 1024, (q + 1) * 1024)
                nc.scalar.activation(fused[:, sl], fraw[:, sl], AF.Relu,
                                     bias=bia3[:], scale=scl3[:],
                                     accum_out=fsum4[:, q:q + 1])
            fsum = sm.tile([128, 1], f32)
            nc.vector.tensor_reduce(fsum[:], fsum4[:], axis=AX.X, op=ALU.add)

            # ------------- gate (applied in the expand copies) ------------
            pg = psE.tile([128, 1024], f32, tag="me")
            nc.tensor.matmul(pg[0:1, 0:1], fsum[:], pp[:, PP_GW:PP_GW + 1],
                             start=True, stop=True)
            u = sm.tile([1, 1], f32)
            nc.scalar.activation(u[:], pg[0:1, 0:1], AF.Exp, bias=0.0,
                                 scale=-1.0)
            nc.vector.tensor_scalar_add(u[:], u[:], 1.0)
            nc.vector.reciprocal(u[:], u[:])
            nc.vector.tensor_scalar(u[:], u[:], 1.0 - GATE_FLOOR, GATE_FLOOR,
                                    ALU.mult, ALU.add)
            pgb = psE.tile([128, 1024], f32, tag="me")
            nc.tensor.matmul(pgb[:, 0:1], ones_t[:], u[:], start=True,
                             stop=True)
            gate_bc = sm.tile([128, 1], f32)
            nc.vector.tensor_copy(gate_bc[:], pgb[:, 0:1])
            if debug:
                nc.scalar.dma_start(dbg["d_fused"].ap(), fused[:])
                nc.scalar.dma_start(dbg["d_small"].ap()[:, 4:5], scl3[:])
                nc.scalar.dma_start(dbg["d_small"].ap()[:, 5:6], bia3[:])
                nc.scalar.dma_start(dbg["d_small"].ap()[:, 9:10], gate_bc[:])

            # ------------- expand + store (deferred) -------------
            # Build the 17 expand items for THIS rep; they are emitted at
            # the EW() weave points of the NEXT rep's mid phases (or right
            # here for the final rep).
            assert not exp_items, f"{len(exp_items)} expand items left over"
            exp_items = _make_expand(fused, gate_bc)
            n_inline = len(exp_items) if rep_i == reps - 1 else 0
            for _ in range(n_inline):
                exp_items.pop(0)()
            cur = nxt

    split_multi_waits(nc)
    return nc


def _get_program():
    if "nc" not in _PROG_CACHE:
        _PROG_CACHE["nc"] = build_program()
    return _PROG_CACHE["nc"]


_PROG_CACHE = {}


def make_in_maps(inputs):
    """Host-side preprocessing: full inputs dict -> per-core in_maps."""
    gi = {k: np.asarray(v, dtype=np.float32) for k, v in inputs.items()}

    x = np.ascontiguousarray(gi["x"].reshape(NCORES, C_IN, S))
    x_bf = x.astype(ml_dtypes.bfloat16)

    Wc = gi["compress_w"]                       # [32, 512]
    K_ = _block_diag(gi["cot_key_w"])           # [32, 32]
    V_ = _block_diag(gi["cot_val_w"])
    A1 = np.zeros((M, 2 * M), np.float32)
    a1 = gi["cot_att1_w"]                       # [4, 8, 16]
    for g in range(G):
        A1[g * MG:(g + 1) * MG, g * 2 * MG:(g + 1) * 2 * MG] = a1[g]
    A1eff = A1[:, :M] @ K_ + A1[:, M:]
    A2 = _block_diag(gi["cot_att2_w"])
    CF = gi["cot_fuse_w"]                       # [32, 32]
    F = gi["fusion_w"]                          # [32, 64]
    Fc, Fct = F[:, :M], F[:, M:]
    M_ = Fct @ CF
    MK = M_ @ K_
    E = gi["expand_w"]                          # [512, 32]

    # compress lhsT [128, 4, 32]: cw[p, kc, c] = Wc[c, kc*128+p]
    WcT = np.ascontiguousarray(Wc.T)            # [512, 32]
    cw = np.zeros((128, 4, 32), np.float32)
    for kc in range(4):
        cw[:, kc, :] = WcT[kc * 128:(kc + 1) * 128, :]
    cw = cw.astype(ml_dtypes.bfloat16)
    # expand lhsT: E.T replicated per chunk block (row tile_position)
    ewt = np.tile(np.ascontiguousarray(E.T), (NCH, 1))   # [128, 512]
    kr = np.zeros((128, 5, 128), np.float32)
    for i, mat in enumerate([A1eff, A2, V_, MK, M_]):
        kr[:, i, :] = _kron128(mat)
    kb = np.zeros((128, 2, 128), np.float32)
    for i, mat in enumerate([Fc, (1.0 - COT_LAM) / 2.0 * M_]):
        kb[:, i, :] = _kron128(mat)
    kb = kb.astype(ml_dtypes.bfloat16)
    jm = np.kron(np.ones((NCH, NCH), np.float32), np.eye(M, dtype=np.float32))
    cm = np.ascontiguousarray(_block_diag(gi["coord_proj_w"]).T)   # [32, 32]
    # pz lhsT [32, 2, 128]: out partitions = 4 chunk replicas of 32 chans
    c2m = np.zeros((32, 2, 128), np.float32)
    c2m[:, 0, :] = np.tile(_block_diag(gi["coord_h_w"]).T, (1, NCH))
    c2m[:, 1, :] = np.tile(_block_diag(gi["coord_wc_w"]).T, (1, NCH))

    pp = np.zeros((128, 8), np.float32)
    rep = lambda v: np.tile(np.asarray(v, np.float32), NCH)
    pp[:, PP_INCG] = rep(gi["inc_g"])
    pp[:, PP_INCB] = rep(gi["inc_b"])
    pp[:, PP_ATTG] = rep(gi["cot_attn_g"])
    pp[:, PP_ATTB] = rep(gi["cot_attn_b"])
    pp[:, PP_FUSG] = rep(gi["fus_g"])
    pp[:, PP_FUSB] = rep(gi["fus_b"])
    pp[:, PP_GW] = rep(gi["gate_w"].reshape(-1) / float(S))

    # coord inorm gamma/beta stacked [h|w] (same params for both halves)
    cp = np.zeros((32, 4), np.float32)
    cp[:, 0] = gi["coord_proj_g"]
    cp[:, 1] = gi["coord_proj_g"]
    cp[:, 2] = gi["coord_proj_b"]
    cp[:, 3] = gi["coord_proj_b"]
    # mean-rescale constants: [alpha*H | beta*W] per partition
    cc = np.zeros((128, 2), np.float32)
    cc[:, 0] = float(gi["coord_alpha"]) * H
    cc[:, 1] = float(gi["coord_beta"]) * W

    on = np.ones((1, 128), np.float32)
    shared = dict(cw=cw, ew=ewt, kr=kr, kb=kb, jm=jm, cm=cm, c2m=c2m,
                  pp=pp, cp=cp, cc=cc, on=on)
    return [dict(shared, x=np.ascontiguousarray(x_bf[i]))
            for i in range(NCORES)]


def kernel(**inputs):
    nc = _get_program()
    in_maps = make_in_maps(inputs)
    res = run_bass_kernel_spmd(nc, in_maps, list(range(NCORES)))
    out = np.stack([np.asarray(res.results[i]["y"])
                    .astype(np.float32).reshape(C_IN, H, W)
                    for i in range(NCORES)], axis=0)
    return out

